# revision 1
# baseline (speedup 1.0000x reference)
"""Trainium2 Bass kernel for CausalSelfAttention (GQA + QK-RMSNorm + RoPE + q_gain).

Sharding: 8-way tensor parallel over query heads. Core c owns q-heads
{2c, 2c+1} and recomputes kv-head c//2 (cheap). Each core produces a
partial output O_c @ Wo_c.T; the host sums the 8 partials.

Self-contained: hardcodes shapes from the problem spec
(x: [1, 4096, 2048], 16 heads / 4 kv heads, head_dim 128).
"""

import os
import numpy as np
from contextlib import ExitStack


def _setup_path():
    try:
        import concourse.bass  # noqa: F401
    except ImportError:
        import sys
        for p in ("/opt/trn_rl_repo", "/root/.axon_site/_ro/trn_rl_repo"):
            if os.path.isdir(p) and p not in sys.path:
                sys.path.insert(0, p)


_setup_path()

import concourse.bass as bass  # noqa: E402
import concourse.bacc as bacc  # noqa: E402
import concourse.mybir as mybir  # noqa: E402
import concourse.tile as tile  # noqa: E402
from concourse.bass_utils import run_bass_kernel_spmd  # noqa: E402

F32 = mybir.dt.float32
F32R = mybir.dt.float32r
BF = mybir.dt.bfloat16
import ml_dtypes
BF_NP = ml_dtypes.bfloat16
ACT = mybir.ActivationFunctionType

T = 4096
D = 2048
HD = 128
KD = D // 128           # 16 contraction tiles
NB = T // 512           # 8 sequence blocks of 512
EPS128 = 128.0 * 1.1920929e-07   # 128 * finfo(f32).eps
NEG = -1.0e30

# Module-level cache of the built program
_NC = None
LAST_RESULT = None


def _r(ap):
    return ap.bitcast(F32R)


def _rope_tables():
    ar = np.arange(0, HD, 2, dtype=np.float32) / np.float32(HD)
    inv = (np.float32(1.0) / (np.float32(10000.0) ** ar)).astype(np.float32)
    t = np.arange(T, dtype=np.float32)
    fr = np.outer(t, inv).astype(np.float32)          # [T, 64]
    cosT = np.ascontiguousarray(np.cos(fr).astype(np.float32).T)  # [64, T]
    sinT = np.ascontiguousarray(np.sin(fr).astype(np.float32).T)
    return cosT, sinT


def _bcast_ap(row_ap, parts=128):
    """Partition-broadcast read AP for a DRAM row of 512 elements."""
    return bass.AP(tensor=row_ap.tensor, offset=row_ap.offset,
                   ap=[[0, parts], [1, 512]])


def _scatter_ap(row_ap):
    """Read AP turning a DRAM row[512] into [128 part, 4]: out[p,j]=row[128j+p]."""
    return bass.AP(tensor=row_ap.tensor, offset=row_ap.offset,
                   ap=[[1, 128], [128, 4]])


def _build():
    nc = bacc.Bacc("TRN2")

    xT = nc.dram_tensor("xT", [D, T], BF, kind="ExternalInput")
    wq = nc.dram_tensor("wq", [128, KD, 256], BF, kind="ExternalInput")
    wk = nc.dram_tensor("wk", [128, KD, 128], BF, kind="ExternalInput")
    wv = nc.dram_tensor("wv", [128, KD, 128], BF, kind="ExternalInput")
    wo = nc.dram_tensor("wo", [128, 2, D], BF, kind="ExternalInput")
    g = nc.dram_tensor("g", [1, 2], F32, kind="ExternalInput")
    y = nc.dram_tensor("y", [T, D], F32, kind="ExternalOutput")

    cosT_np, sinT_np = _rope_tables()
    cos2_np = np.ascontiguousarray(np.concatenate([cosT_np, cosT_np], axis=0))
    sin2_np = np.ascontiguousarray(np.concatenate([sinT_np, sinT_np], axis=0))
    cos_d = nc.inline_tensor(cos2_np, "cosT")
    sin_d = nc.inline_tensor(sin2_np, "sinT")
    tri_np = np.where(np.arange(512)[None, :] >= np.arange(128)[:, None],
                      np.float32(0.0), np.float32(NEG)).astype(np.float32)
    tri_d = nc.inline_tensor(tri_np, "tri")          # [128, 512] mask for diag
    ones_d = nc.inline_tensor(np.ones((128, 1), BF_NP), "ones")
    id_d = nc.inline_tensor(np.eye(128).astype(BF_NP), "ident")

    with tile.TileContext(nc) as tc, ExitStack() as ctx:
        consts = ctx.enter_context(tc.tile_pool(name="consts", bufs=1))
        persist = ctx.enter_context(tc.tile_pool(name="persist", bufs=1))
        xt_pool = ctx.enter_context(tc.tile_pool(name="xt", bufs=18))
        qt_pool = ctx.enter_context(tc.tile_pool(name="qtp", bufs=3))
        sq_pool = ctx.enter_context(tc.tile_pool(name="sqp", bufs=3))
        stage_pool = ctx.enter_context(tc.tile_pool(name="stg", bufs=6))
        rt_pool = ctx.enter_context(tc.tile_pool(name="rtp", bufs=6))
        p_pool = ctx.enter_context(tc.tile_pool(name="ppool", bufs=6))
        o_pool = ctx.enter_context(tc.tile_pool(name="opool", bufs=3))
        y_pool = ctx.enter_context(tc.tile_pool(name="ypool", bufs=3))
        row_pool = ctx.enter_context(tc.tile_pool(name="rowp", bufs=12))
        bc_pool = ctx.enter_context(tc.tile_pool(name="bcp", bufs=6))
        vt_pool = ctx.enter_context(tc.tile_pool(name="vtp", bufs=2))
        cs_pool = ctx.enter_context(tc.tile_pool(name="csp", bufs=4))
        ps_proj = ctx.enter_context(tc.tile_pool(name="psproj", bufs=3, space="PSUM"))
        ps_st = ctx.enter_context(tc.tile_pool(name="psst", bufs=3, space="PSUM"))
        ps_pv = ctx.enter_context(tc.tile_pool(name="pspv", bufs=1, space="PSUM"))
        ps_sum = ctx.enter_context(tc.tile_pool(name="pssum", bufs=1, space="PSUM"))
        dram = ctx.enter_context(tc.tile_pool(name="dramp", bufs=1, space="DRAM"))

        # Resident constants
        wq_sb = consts.tile([128, KD, 256], BF, tag="wq")
        nc.sync.dma_start(out=wq_sb, in_=wq[:])
        wk_sb = consts.tile([128, KD, 128], BF, tag="wk")
        nc.sync.dma_start(out=wk_sb, in_=wk[:])
        wv_sb = consts.tile([128, KD, 128], BF, tag="wv")
        nc.sync.dma_start(out=wv_sb, in_=wv[:])
        wo_sb = consts.tile([128, 2, D], BF, tag="wo")
        nc.sync.dma_start(out=wo_sb, in_=wo[:])
        g_sb = consts.tile([1, 2], F32, tag="g")
        nc.sync.dma_start(out=g_sb, in_=g[:])
        tri_sb = consts.tile([128, 512], F32, tag="tri")
        nc.sync.dma_start(out=tri_sb, in_=tri_d[:])
        ones_sb = consts.tile([128, 1], BF, tag="ones")
        nc.sync.dma_start(out=ones_sb, in_=ones_d[:])
        id_sb = consts.tile([128, 128], BF, tag="id")
        nc.sync.dma_start(out=id_sb, in_=id_d[:])

        rows = dram.tile([NB, 5, 512], F32, tag="rows")

        kt_tiles = []
        v_tiles = []
        rk_tiles = []
        qt_tiles = [None] * NB
        o_tiles = [None] * NB
        for b in range(NB):
            kt_tiles.append(persist.tile([128, 512], BF, tag=f"kt{b}", name=f"kt{b}"))
            v_tiles.append(persist.tile([128, 4, 128], BF, tag=f"v{b}", name=f"v{b}"))
            rk_tiles.append(persist.tile([128, 4], F32, tag=f"rk{b}", name=f"rk{b}"))

        for b in range(NB):
            t0 = b * 512
            tsl = slice(t0, t0 + 512)

            # ---- Phase A: QKV projections (transposed orientation) ----
            xts = []
            for k in range(KD):
                xt = xt_pool.tile([128, 512], BF, tag="xt")
                nc.sync.dma_start(out=xt, in_=xT[k * 128:(k + 1) * 128, tsl])
                xts.append(xt)
            qa_ps = ps_proj.tile([128, 512], F32, tag="proj")
            qb_ps = ps_proj.tile([128, 512], F32, tag="proj")
            for k in range(KD):
                st_, sp_ = (k == 0), (k == KD - 1)
                nc.tensor.matmul(qa_ps, wq_sb[:, k, 0:128], xts[k],
                                 start=st_, stop=sp_)
                nc.tensor.matmul(qb_ps, wq_sb[:, k, 128:256], xts[k],
                                 start=st_, stop=sp_)
            kt_ps = ps_proj.tile([128, 512], F32, tag="proj")
            vt_ps = ps_proj.tile([128, 512], F32, tag="proj")
            for k in range(KD):
                st_, sp_ = (k == 0), (k == KD - 1)
                nc.tensor.matmul(kt_ps, wk_sb[:, k, :], xts[k],
                                 start=st_, stop=sp_)
                nc.tensor.matmul(vt_ps, wv_sb[:, k, :], xts[k],
                                 start=st_, stop=sp_)

            # Stage proj psums to SBUF so the psum slots free early and
            # block b+1's projections overlap block b's norm/rope chain.
            qa_st = stage_pool.tile([128, 512], F32, tag="stg")
            nc.scalar.copy(qa_st, qa_ps)
            qb_st = stage_pool.tile([128, 512], F32, tag="stg")
            nc.vector.tensor_copy(qb_st, qb_ps)
            kt_st = stage_pool.tile([128, 512], F32, tag="stg")
            nc.scalar.copy(kt_st, kt_ps)

            # ---- RMS-norm row factors ----
            # rq = gain / sqrt(ssq + 128*eps)   (scale 128^-0.5 cancels exactly)
            # rk = sqrt(128 / (ssq + 128*eps))
            rq_bc = [None, None]
            for ri, ps, kind, h in ((0, qa_st, 'q', 0), (1, qb_st, 'q', 1),
                                    (2, kt_st, 'k', None)):
                sq = sq_pool.tile([128, 512], BF, tag="sq")
                nc.scalar.activation(sq, ps, ACT.Square)
                ssq = ps_proj.tile([1, 512], F32, tag="proj")
                nc.tensor.matmul(ssq, ones_sb, sq)
                row = row_pool.tile([1, 512], F32, tag="row")
                nc.vector.tensor_scalar_add(row, ssq, EPS128)
                rec = row_pool.tile([1, 512], F32, tag="row")
                nc.vector.reciprocal(rec, row)
                fin = row_pool.tile([1, 512], F32, tag="row")
                if kind == 'q':
                    nc.scalar.activation(fin, rec, ACT.Sqrt)
                    fin2 = row_pool.tile([1, 512], F32, tag="row")
                    nc.vector.tensor_scalar_mul(fin2, fin, g_sb[:, h:h + 1])
                    nc.sync.dma_start(out=rows[b, ri], in_=fin2)
                    bc = bc_pool.tile([128, 512], F32, tag="bc")
                    nc.sync.dma_start(out=bc, in_=_bcast_ap(rows[b, ri]))
                    rq_bc[h] = bc
                else:
                    nc.scalar.activation(fin, rec, ACT.Sqrt, scale=128.0)
                    nc.sync.dma_start(out=rows[b, ri], in_=fin)
                    nc.sync.dma_start(out=rk_tiles[b], in_=_scatter_ap(rows[b, ri]))

            # ---- RoPE + rq application ----
            cos_b = cs_pool.tile([128, 512], F32, tag="cos")
            nc.sync.dma_start(out=cos_b, in_=cos_d[:, tsl])
            sin_b = cs_pool.tile([128, 512], F32, tag="sin")
            nc.sync.dma_start(out=sin_b, in_=sin_d[:, tsl])

            def rope(dst, src):
                # SB+SB operand pairs must share base partition; cos_b/sin_b
                # carry the table duplicated in both halves.
                t1 = rt_pool.tile([64, 512], F32, tag="rt")
                t2 = rt_pool.tile([64, 512], F32, tag="rt")
                nc.vector.tensor_mul(t1, src[0:64], cos_b[0:64])
                nc.vector.tensor_mul(t2, src[64:128], sin_b[64:128])
                nc.vector.tensor_add(dst[0:64], t1, t2)
                t3 = rt_pool.tile([64, 512], F32, tag="rt")
                t4 = rt_pool.tile([64, 512], F32, tag="rt")
                nc.vector.tensor_mul(t3, src[64:128], cos_b[64:128])
                nc.vector.tensor_mul(t4, src[0:64], sin_b[0:64])
                nc.vector.tensor_sub(dst[64:128], t3, t4)

            qt = qt_pool.tile([128, 2, 512], BF, tag="qt")
            qt_tiles[b] = qt
            for h in (0, 1):
                qtf = sq_pool.tile([128, 512], F32, tag="qtf")
                rope(qtf, qa_st if h == 0 else qb_st)
                nc.vector.tensor_mul(qt[:, h, :], qtf, rq_bc[h])
            rope(kt_tiles[b], kt_st)

            # ---- V transpose to [tk, hd] via PE ----
            vt_sb = vt_pool.tile([128, 512], BF, tag="vt")
            nc.vector.tensor_copy(vt_sb, vt_ps)
            for jj in range(4):
                tp = ps_st.tile([128, 128], BF, tag="st")
                nc.tensor.transpose(tp, vt_sb[:, jj * 128:(jj + 1) * 128], id_sb)
                nc.vector.tensor_copy(v_tiles[b][:, jj, :], tp)

            # ---- Phase B: attention for block b (both heads) ----
            o_sb = o_pool.tile([128, 2, 512], BF, tag="o")
            o_tiles[b] = o_sb
            nk = 4 * (b + 1)
            for h in (0, 1):
                pv_ps = ps_pv.tile([128, 512], F32, tag="pv")
                sum_ps = ps_sum.tile([1, 512], F32, tag="sums")
                qh = qt[:, h, :]
                for j in range(nk):
                    kb, ko = j // 4, (j % 4) * 128
                    st = ps_st.tile([128, 512], F32, tag="st")
                    nc.tensor.matmul(st, kt_tiles[kb][:, ko:ko + 128], qh)
                    p = p_pool.tile([128, 512], BF, tag="p")
                    scale_ap = rk_tiles[kb][:, (j % 4):(j % 4) + 1]
                    if j >= 4 * b:      # diagonal tile: causal mask
                        off = (j - 4 * b) * 128
                        nc.vector.tensor_add(st[:, off:off + 128],
                                             st[:, off:off + 128],
                                             tri_sb[:, 0:128])
                        if off:
                            nc.vector.memset(p[:, 0:off], 0.0)
                        nc.scalar.activation(p[:, off:512], st[:, off:512],
                                             ACT.Exp, scale=scale_ap)
                    else:
                        nc.scalar.activation(p, st, ACT.Exp, scale=scale_ap)
                    nc.tensor.matmul(pv_ps, v_tiles[kb][:, j % 4, :], p,
                                     start=(j == 0), stop=(j == nk - 1))
                    nc.tensor.matmul(sum_ps, ones_sb, p,
                                     start=(j == 0), stop=(j == nk - 1))
                rs = row_pool.tile([1, 512], F32, tag="row")
                nc.vector.reciprocal(rs, sum_ps)
                nc.sync.dma_start(out=rows[b, 3 + h], in_=rs)
                rs_bc = bc_pool.tile([128, 512], F32, tag="bc")
                nc.sync.dma_start(out=rs_bc, in_=_bcast_ap(rows[b, 3 + h]))
                nc.vector.tensor_mul(o_sb[:, h, :], pv_ps, rs_bc)

            # ---- Phase C: output projection (partial Y) ----
            for t4 in range(4):
                y_sb = y_pool.tile([128, D], F32, tag="y")
                for oc in range(4):
                    y_ps = ps_st.tile([128, 512], F32, tag="st")
                    for h in (0, 1):
                        nc.tensor.matmul(
                            y_ps,
                            o_sb[:, h, t4 * 128:(t4 + 1) * 128],
                            wo_sb[:, h, oc * 512:(oc + 1) * 512],
                            start=(h == 0), stop=(h == 1))
                    if oc % 2 == 0:
                        nc.vector.tensor_copy(y_sb[:, oc * 512:(oc + 1) * 512], y_ps)
                    else:
                        nc.scalar.copy(y_sb[:, oc * 512:(oc + 1) * 512], y_ps)
                nc.sync.dma_start(
                    out=y[t0 + t4 * 128: t0 + (t4 + 1) * 128, :], in_=y_sb)

    nc.finalize()
    return nc


def _get_nc():
    global _NC
    if _NC is None:
        _NC = _build()
    return _NC


def kernel(x, Wq, Wk, Wv, Wo, q_gain):
    global LAST_RESULT
    x = np.asarray(x, dtype=np.float32)
    Wq = np.asarray(Wq, dtype=np.float32)
    Wk = np.asarray(Wk, dtype=np.float32)
    Wv = np.asarray(Wv, dtype=np.float32)
    Wo = np.asarray(Wo, dtype=np.float32)
    q_gain = np.asarray(q_gain, dtype=np.float32)

    xT = np.ascontiguousarray(x[0].T).astype(BF_NP)         # [D, T]
    in_maps = []
    for c in range(8):
        h0, kv = 2 * c, c // 2
        wq_c = Wq[h0 * 128:(h0 + 2) * 128]                   # [256, D]
        wq_in = np.ascontiguousarray(
            wq_c.T.reshape(KD, 128, 256).transpose(1, 0, 2)).astype(BF_NP)
        wk_c = Wk[kv * 128:(kv + 1) * 128]                   # [128, D]
        wk_in = np.ascontiguousarray(
            wk_c.T.reshape(KD, 128, 128).transpose(1, 0, 2)).astype(BF_NP)
        wv_c = Wv[kv * 128:(kv + 1) * 128]
        wv_in = np.ascontiguousarray(
            wv_c.T.reshape(KD, 128, 128).transpose(1, 0, 2)).astype(BF_NP)
        wo_c = Wo[:, h0 * 128:(h0 + 2) * 128]                # [D, 256]
        wo_in = np.ascontiguousarray(
            wo_c.T.reshape(2, 128, D).transpose(1, 0, 2)).astype(BF_NP)
        g_in = np.ascontiguousarray(q_gain[h0:h0 + 2].reshape(1, 2))
        in_maps.append({"xT": xT, "wq": wq_in, "wk": wk_in, "wv": wv_in,
                        "wo": wo_in, "g": g_in})

    trace = bool(int(os.environ.get("KER_TRACE", "0")))
    res = run_bass_kernel_spmd(_get_nc(), in_maps, list(range(8)), trace=trace)
    LAST_RESULT = res
    acc = np.zeros((T, D), np.float64)
    for c in range(8):
        acc += res.results[c]["y"]
    return acc.astype(np.float32).reshape(1, T, D)

